# revision 2
# baseline (speedup 1.0000x reference)
"""Trainium2 Bass kernel for a 2-layer bidirectional tanh-RNN.

B=32, S=1024, I=1024, H=512, L=2 (input_size == 2*H).

Sharding: 8 NeuronCores = 2 direction roles x 4 batch shards (8 sequences per
core). Cores 0-3 run the forward direction, cores 4-7 the reverse direction
(fed time-reversed X and the reverse weight set; the program is
role-agnostic). The sequence dim cannot be sharded (recurrence), so each core
carries a full 1024-step serial chain per layer; splitting the two
independent directions across core groups halves the serial TensorE work per
core versus running both directions on every core.

Per-core design:
  - transposed state layout h^T: [128 partitions = H-row-within-chunk,
    cols = t*(HC*b) + m*b + i] (m = H chunk 0..3, i = batch 0..7), fp16,
    fp32 PSUM accumulation
  - x-projections precomputed blockwise as dense matmuls (weights stationary,
    x^T streamed, N=512), bias folded into the PSUM->SBUF evacuation
  - recurrence step: one identity-matmul injects xp_t into PSUM, then 16
    accumulating 128x128 matmuls apply W_hh; tanh on ScalarE writes h_t back
    to SBUF. Weight-load bandwidth bounds the step (~1us/step).
  - between layers, core pairs (i, i+4) exchange their layer-0 hidden
    sequences via a pairwise AllGather (HBM bounce). Role asymmetry (which
    AG slot is the partner and which Wih1 column block applies) is resolved
    in data: both AG slots are consumed time-mirrored with separate weight
    sets, one of which the host zeroes.
  - sequence reversal is entirely compile-time index mirroring + one DVE
    reversed copy per exchange block; no negative-stride DMA.
"""

import numpy as np

import concourse.bass as bass
import concourse.mybir as mybir
import concourse.tile as tile
from concourse import bacc
from concourse.bass_utils import run_bass_kernel_spmd
from concourse.masks import make_identity

F16 = mybir.dt.float16
F32 = mybir.dt.float32
Tanh = mybir.ActivationFunctionType.Tanh

HC = 4    # H chunks (H = 512)
IC = 8    # I chunks (I = 1024)
S = 1024
BSZ = 8       # batch per core
N_SHARDS = 4  # batch shards per direction role
N_CORES = 8


def _build(nc, b=BSZ, TB=64):
    G = HC * b  # 32 cols per timestep

    xT = nc.dram_tensor("xT", [IC, 128, S, b], F16, kind="ExternalInput").ap()
    wih0 = nc.dram_tensor("wih0", [IC, HC, 128, 128], F16, kind="ExternalInput").ap()
    whh = nc.dram_tensor("whh", [2, HC, HC, 128, 128], F16, kind="ExternalInput").ap()
    wih1own = nc.dram_tensor("wih1own", [HC, HC, 128, 128], F16, kind="ExternalInput").ap()
    wih1s = nc.dram_tensor("wih1s", [2, HC, HC, 128, 128], F16, kind="ExternalInput").ap()
    bias = nc.dram_tensor("bias", [128, 8], F32, kind="ExternalInput").ap()
    ys1 = nc.dram_tensor("ys1", [128, S, HC, b], F16, kind="ExternalOutput").ap()
    hid = nc.dram_tensor("hid", [2, 128, G], F16, kind="ExternalOutput").ap()

    with tile.TileContext(nc) as tc:
        import contextlib
        ctx = contextlib.ExitStack()
        with ctx:
            const_pool = ctx.enter_context(tc.tile_pool(name="const", bufs=1))
            w_pool = ctx.enter_context(tc.tile_pool(name="w", bufs=1))
            ys0_pool = ctx.enter_context(tc.tile_pool(name="ys0", bufs=1))
            xp_pool = ctx.enter_context(tc.tile_pool(name="xp", bufs=1))
            xblk_pool = ctx.enter_context(tc.tile_pool(name="xblk", bufs=2))
            stage_pool = ctx.enter_context(tc.tile_pool(name="stage", bufs=1))
            ring_pool = ctx.enter_context(tc.tile_pool(name="ring", bufs=2))
            dram_pool = ctx.enter_context(tc.tile_pool(name="dram", bufs=1, space="DRAM"))
            ps_rec = ctx.enter_context(tc.tile_pool(name="psrec", bufs=3, space="PSUM"))
            ps_xp = ctx.enter_context(tc.tile_pool(name="psxp", bufs=2, space="PSUM"))

            ident = const_pool.tile([128, 128], F16, tag="ident")
            make_identity(nc, ident[:])
            bias_sb = const_pool.tile([128, 8], F32, tag="bias")
            nc.sync.dma_start(bias_sb[:], bias[:])

            ys0 = ys0_pool.tile([128, S * G], F16, tag="ys0")

            whh_sb = w_pool.tile([128, 2 * HC * HC * 128], F16, tag="whh")
            for l in range(2):
                for j in range(HC):
                    for m in range(HC):
                        k = (l * HC + j) * HC + m
                        nc.sync.dma_start(
                            whh_sb[:, k * 128:(k + 1) * 128], whh[l, j, m])
            whh_t = lambda l, j, m: whh_sb[:, ((l * HC + j) * HC + m) * 128:
                                           ((l * HC + j) * HC + m + 1) * 128]
            wih0_sb = w_pool.tile([128, IC * HC * 128], F16, tag="wih")
            for c in range(IC):
                for m in range(HC):
                    k = c * HC + m
                    nc.sync.dma_start(
                        wih0_sb[:, k * 128:(k + 1) * 128], wih0[c, m])
            wih0_t = lambda c, m: wih0_sb[:, (c * HC + m) * 128:
                                          (c * HC + m + 1) * 128]
            wih1_sb = w_pool.tile([128, 3 * HC * HC * 128], F16, tag="wih")
            for s_ in range(3):
                for c in range(HC):
                    for m in range(HC):
                        k = (s_ * HC + c) * HC + m
                        src = wih1own[c, m] if s_ == 0 else wih1s[s_ - 1, c, m]
                        nc.sync.dma_start(
                            wih1_sb[:, k * 128:(k + 1) * 128], src)
            wih1_t = lambda s_, c, m: wih1_sb[:, ((s_ * HC + c) * HC + m) * 128:
                                              ((s_ * HC + c) * HC + m + 1) * 128]

            # ---- layer 0 xproj (own direction, local time order) ----
            xp0 = xp_pool.tile([128, S * G], F16, tag="xp")
            for blk in range(S // TB):
                t0 = blk * TB
                xb = xblk_pool.tile([128, IC * TB * b], F16, tag="xblk",
                                    name=f"xb_{blk}")
                for c in range(IC):
                    nc.sync.dma_start(
                        xb[:, c * TB * b:(c + 1) * TB * b],
                        xT[c, :, t0:t0 + TB, :])
                for m in range(HC):
                    ps = ps_xp.tile([128, TB * b], F32, tag="psxp",
                                    name=f"psxp0_{blk}_{m}")
                    for c in range(IC):
                        nc.tensor.matmul(
                            ps[:], wih0_t(c, m),
                            xb[:, c * TB * b:(c + 1) * TB * b],
                            start=(c == 0), stop=(c == IC - 1))
                    xpv = xp0[:].rearrange("p (t g) -> p t g", g=G)
                    nc.vector.tensor_scalar_add(
                        xpv[:, t0:t0 + TB, m * b:(m + 1) * b], ps[:],
                        bias_sb[:, m:m + 1])

            def rec_step(ps, wl, xp, xp_col, hprev):
                nc.tensor.matmul(
                    ps[:], ident[:], xp[:, xp_col * G:(xp_col + 1) * G],
                    start=True, stop=(hprev is None), skip_group_check=True)
                if hprev is not None:
                    for j in range(HC):
                        for m in range(HC):
                            nc.tensor.matmul(
                                ps[:, m * b:(m + 1) * b], whh_t(wl, j, m),
                                hprev[:, j * b:(j + 1) * b],
                                start=False, stop=(j == HC - 1),
                                skip_group_check=True)

            # ---- layer 0 recurrence ----
            for t in range(S):
                ps = ps_rec.tile([128, G], F32, tag="psr", name=f"ps0_{t}")
                hprev = None if t == 0 else ys0[:, (t - 1) * G:t * G]
                rec_step(ps, 0, xp0, t, hprev)
                nc.scalar.activation(ys0[:, t * G:(t + 1) * G], ps[:], Tanh)
            nc.sync.dma_start(hid[0], ys0[:, (S - 1) * G:S * G])

            # ---- exchange: pairwise AllGather of ys0 ----
            bin_ = dram_pool.tile([128, S * G], F16, tag="bin")
            bout = dram_pool.tile([2, 128, S * G], F16, tag="bout")
            nc.sync.dma_start(bin_[:], ys0[:])
            nc.gpsimd.collective_compute(
                "AllGather", mybir.AluOpType.bypass,
                replica_groups=[[0, 4], [1, 5], [2, 6], [3, 7]],
                ins=[bin_.opt()], outs=[bout.opt()])

            # ---- layer 1 xproj: own half + mirrored partner (both AG slots,
            # one weight set zeroed host-side) ----
            xp1 = xp_pool.tile([128, S * G], F16, tag="xp")
            for blk in range(S // TB):
                t0 = blk * TB
                sg = stage_pool.tile([128, 2 * TB * G], F16, tag="sg",
                                     name=f"sg_{blk}")
                sgr = stage_pool.tile([128, 2 * TB * G], F16, tag="sgr",
                                      name=f"sgr_{blk}")
                p0 = (S - t0 - TB) * G
                for s_ in range(2):
                    nc.sync.dma_start(
                        sg[:, s_ * TB * G:(s_ + 1) * TB * G],
                        bout[s_, :, p0:p0 + TB * G])
                sgv = sg[:].rearrange("p (s t g) -> p s t g", s=2, g=G)
                sgrv = sgr[:].rearrange("p (s t g) -> p s t g", s=2, g=G)
                nc.vector.tensor_copy(sgrv[:], sgv[:, :, ::-1, :])
                ys0v = ys0[:].rearrange("p (t g) -> p t g", g=G)
                for m in range(HC):
                    ps = ps_xp.tile([128, TB * b], F32, tag="psxp",
                                    name=f"psxp1_{blk}_{m}")
                    for c in range(HC):
                        nc.tensor.matmul(
                            ps[:], wih1_t(0, c, m),
                            ys0v[:, t0:t0 + TB, c * b:(c + 1) * b],
                            start=(c == 0), stop=False)
                    for s_ in range(2):
                        for c in range(HC):
                            nc.tensor.matmul(
                                ps[:], wih1_t(1 + s_, c, m),
                                sgrv[:, s_, :, c * b:(c + 1) * b],
                                start=False, stop=(s_ == 1 and c == HC - 1))
                    xpv = xp1[:].rearrange("p (t g) -> p t g", g=G)
                    nc.vector.tensor_scalar_add(
                        xpv[:, t0:t0 + TB, m * b:(m + 1) * b], ps[:],
                        bias_sb[:, HC + m:HC + m + 1])

            # ---- layer 1 recurrence (ring buffered, DMA out) ----
            rings = [None, None]
            for blk in range(S // TB):
                t0 = blk * TB
                rings[0], rings[1] = rings[1], ring_pool.tile(
                    [128, TB * G], F16, tag="ring", name=f"ring_{blk}")
                ring = rings[1]
                for tt in range(TB):
                    t = t0 + tt
                    ps = ps_rec.tile([128, G], F32, tag="psr", name=f"ps1_{t}")
                    if t == 0:
                        hprev = None
                    elif tt == 0:
                        hprev = rings[0][:, (TB - 1) * G:TB * G]
                    else:
                        hprev = ring[:, (tt - 1) * G:tt * G]
                    rec_step(ps, 1, xp1, t, hprev)
                    nc.scalar.activation(
                        ring[:, tt * G:(tt + 1) * G], ps[:], Tanh)
                nc.sync.dma_start(ys1[:, t0:t0 + TB], ring[:])
                if blk == S // TB - 1:
                    nc.sync.dma_start(hid[1], ring[:, (TB - 1) * G:TB * G])

    return nc


_NC_CACHE = {}


def _get_nc():
    if "nc" not in _NC_CACHE:
        nc = bacc.Bacc("TRN2", target_bir_lowering=False, debug=False,
                       num_devices=N_CORES)
        _build(nc)
        nc.compile()
        _NC_CACHE["nc"] = nc
    return _NC_CACHE["nc"]


def _prep_core_inputs(core, X, W_ih, W_hh, b_, W_ih_r, W_hh_r, b_r):
    role = core // N_SHARDS  # 0 fwd, 1 rev
    shard = core % N_SHARDS
    Xs = X[shard * BSZ:(shard + 1) * BSZ]
    if role == 1:
        Xs = Xs[:, ::-1]
    xT = np.ascontiguousarray(
        Xs.transpose(2, 1, 0).reshape(IC, 128, S, BSZ)).astype(np.float16)

    Wih = (W_ih if role == 0 else W_ih_r)
    Whh = (W_hh if role == 0 else W_hh_r)
    bb = (b_ if role == 0 else b_r)

    def tiles(W, nch):
        return np.ascontiguousarray(
            W.T.reshape(nch, 128, HC, 128).transpose(0, 2, 1, 3)).astype(np.float16)

    wih0 = tiles(Wih[0], IC)
    whht = np.stack([tiles(Whh[0], HC), tiles(Whh[1], HC)])
    if role == 0:
        w_own = tiles(Wih[1][:, :512], HC)
        w_partner = tiles(Wih[1][:, 512:], HC)
        wih1s = np.stack([np.zeros_like(w_partner), w_partner])
    else:
        w_own = tiles(Wih[1][:, 512:], HC)
        w_partner = tiles(Wih[1][:, :512], HC)
        wih1s = np.stack([w_partner, np.zeros_like(w_partner)])
    bias = np.zeros((128, 8), np.float32)
    bias[:, :HC] = bb[0].reshape(HC, 128).T
    bias[:, HC:] = bb[1].reshape(HC, 128).T
    return {"xT": xT, "wih0": wih0, "whh": whht, "wih1own": w_own,
            "wih1s": wih1s, "bias": bias}


def _assemble(results):
    B = BSZ * N_SHARDS
    out = np.empty((B, S, 1024), np.float32)
    hid = np.empty((4, B, 512), np.float32)
    for core, res in enumerate(results):
        role = core // N_SHARDS
        shard = core % N_SHARDS
        sl = slice(shard * BSZ, (shard + 1) * BSZ)
        ys1 = res["ys1"].astype(np.float32)  # [128, S, HC, b] local order
        if role == 1:
            ys1 = ys1[:, ::-1]
        col = slice(0, 512) if role == 0 else slice(512, 1024)
        out[sl, :, col] = ys1.transpose(3, 1, 2, 0).reshape(BSZ, S, 512)
        h = res["hid"].astype(np.float32)  # [2, 128, G]
        for l in range(2):
            hid[2 * l + role, sl] = \
                h[l].reshape(128, HC, BSZ).transpose(2, 1, 0).reshape(BSZ, 512)
    return out, hid


def kernel(X, W_ih, W_hh, b, W_ih_r, W_hh_r, b_r):
    X = np.asarray(X, np.float32)
    W_ih = np.asarray(W_ih, np.float32)
    W_hh = np.asarray(W_hh, np.float32)
    b = np.asarray(b, np.float32)
    W_ih_r = np.asarray(W_ih_r, np.float32)
    W_hh_r = np.asarray(W_hh_r, np.float32)
    b_r = np.asarray(b_r, np.float32)

    nc = _get_nc()
    in_maps = [
        _prep_core_inputs(c, X, W_ih, W_hh, b, W_ih_r, W_hh_r, b_r)
        for c in range(N_CORES)
    ]
    res = run_bass_kernel_spmd(nc, in_maps, core_ids=list(range(N_CORES)))
    return _assemble([res.results[c] for c in range(N_CORES)])
